# revision 36
# baseline (speedup 1.0000x reference)
"""Trainium2 Bass kernel for nn_MCPHiddenLayers (dense MLP mixture-of-primitives).

Data-parallel over 8 NeuronCores: batch 32768 is split into 8 shards of
4096 rows; all MLP parameters are replicated. Each core runs an identical
NEFF built with the Tile framework.

Layout strategy: activations are kept feature-major ("transposed",
[feat_part, batch_free]) so every layer's contraction dim lands on SBUF
partitions with no per-layer transposes; the input features are
transposed once on-chip via the PE array. Matmuls run in a reduced
precision dtype (float32r by default: single-pass fp32, ~tf32 accuracy)
with fp32 PSUM accumulation. The final primitive layer and the value-net
output layer run "flipped" (batch on PSUM partitions) so the Gaussian
mixture can use per-partition scalar broadcasts; their free-dim biases
are injected via a ones-row matmul that also initializes the PSUM bank.
"""
import os
import sys

sys.path.insert(0, "/opt/trn_rl_repo")

import numpy as np

import concourse.bacc as bacc
import concourse.mybir as mybir
import concourse.tile as tile
from concourse.bass_utils import run_bass_kernel_spmd
from concourse.masks import make_identity

# ---- problem shapes (hardcoded from the spec) ----
B = 32768
S = 512
G = 128
A = 32
NP = 16          # num_primitives
H1, H2 = 512, 256
VH1, VH2 = 1024, 512
F = S + G        # 640

NCORES = 8
BC = B // NCORES  # 4096 rows per core
P = 128

# ---- tuning knobs ----
NT = int(os.environ.get("KNT", "512"))        # batch tile (matmul moving dim)
_DT = os.environ.get("KDT", "f16")            # matmul operand dtype
NTILES = BC // NT
NB = NT // P      # batch chunks of 128 per tile

F32 = mybir.dt.float32
DT = {"f32r": mybir.dt.float32r, "bf16": mybir.dt.bfloat16,
      "f16": mybir.dt.float16, "f32": mybir.dt.float32}[_DT]
DT2 = mybir.dt.size(DT) == 2   # 2-byte path: host-cast features + DMA transpose
AF = mybir.ActivationFunctionType
ALU = mybir.AluOpType

_NC_CACHE = []


def build():
    nc = bacc.Bacc(None, target_bir_lowering=False)

    feat_d = nc.dram_tensor(
        "features", [BC, F], DT if DT2 else F32, kind="ExternalInput"
    )
    w = {}
    for name, shape in [
        ("se_w1", [S, H1]), ("se_b1", [H1]), ("se_w2", [H1, H2]), ("se_b2", [H2]),
        ("ge_w1", [G, H1]), ("ge_b1", [H1]), ("ge_w2", [H1, H2]), ("ge_b2", [H2]),
        ("gt_w1", [2 * H2, H2]), ("gt_b1", [H2]), ("gt_w2", [H2, NP]), ("gt_b2", [NP]),
        ("pse_w1", [S, H1]), ("pse_b1", [H1]), ("pse_w2", [H1, H2]), ("pse_b2", [H2]),
        ("pw1", [NP, H2, H2]), ("pb1", [NP, H2]),
        ("pw2", [NP, H2, 2 * A]), ("pb2", [NP, 2 * A]),
        ("vf_w1", [F, VH1]), ("vf_b1", [VH1]), ("vf_w2", [VH1, VH2]), ("vf_b2", [VH2]),
    ]:
        is_wt = name[-1].isdigit() and "_b" not in name and not name.startswith("pb")
        w[name] = nc.dram_tensor(
            name, shape, DT if (DT2 and is_wt) else F32, kind="ExternalInput"
        )
    mean_d = nc.dram_tensor("mean", [BC, A], F32, kind="ExternalOutput")
    value_d = nc.dram_tensor("value", [BC, VH2], F32, kind="ExternalOutput")

    big_bufs = 4 if DT != F32 and mybir.dt.size(DT) == 2 else 2
    emb_bufs = 4 if mybir.dt.size(DT) == 2 else 3

    with tile.TileContext(nc) as tc:
        with (
            tc.tile_pool(name="wpool", bufs=1) as wpool,
            tc.tile_pool(name="apool", bufs=2) as apool,
            tc.tile_pool(name="ptp", bufs=1 if DT2 else 2, space="PSUM") as pp_tp,
            tc.tile_pool(name="pmm", bufs=7 if DT2 else 3, space="PSUM") as pp_mm,
            tc.tile_pool(name="pflip", bufs=1, space="PSUM") as pp_flip,
        ):
            # ---------- static setup ----------
            ident_f = wpool.tile([P, P], F32)
            make_identity(nc, ident_f)
            ident = wpool.tile([P, P], DT)
            nc.vector.tensor_copy(ident[:], ident_f[:])

            wdma = nc.sync.dma_start if DT2 else nc.gpsimd.dma_start

            def load_w(name, rows, cols, dma=None):
                kc = rows // P
                sb = wpool.tile([P, kc * cols], DT, name=name)
                (dma or wdma)(
                    sb[:].rearrange("p (c m) -> p c m", c=kc),
                    w[name][:].rearrange("(c p) m -> p c m", p=P),
                )
                return sb

            def load_b(name, n):
                c = n // P
                sb = wpool.tile([P, c], F32, name=name)
                nc.gpsimd.dma_start(sb[:], w[name][:].rearrange("(c p) -> p c", p=P))
                return sb

            _cache = {}

            def WT(name, rows, cols, dma=None):
                if name not in _cache:
                    _cache[name] = load_w(name, rows, cols, dma)
                return _cache[name]

            def BCOL(name, n):
                key = "b:" + name
                if key not in _cache:
                    _cache[key] = load_b(name, n)
                return _cache[key]

            kcp = H2 // P  # 2

            def PW():
                if "pw" not in _cache:
                    pw1 = wpool.tile([P, NP * kcp * H2], DT, name="pw1")
                    pw2 = wpool.tile([P, NP * kcp * 2 * A], DT, name="pw2")
                    wdma(
                        pw1[:].rearrange("p (q c m) -> p q c m", q=NP, c=kcp),
                        w["pw1"][:].rearrange("q (c p) m -> p q c m", p=P),
                    )
                    wdma(
                        pw2[:].rearrange("p (q c m) -> p q c m", q=NP, c=kcp),
                        w["pw2"][:].rearrange("q (c p) m -> p q c m", p=P),
                    )
                    _cache["pw"] = (pw1, pw2)
                return _cache["pw"]

            def GTB2():
                if "gt_b2" not in _cache:
                    gt_b2 = wpool.tile([NP, 1], F32, name="gt_b2")
                    nc.sync.dma_start(gt_b2[:], w["gt_b2"][:].unsqueeze(1))
                    _cache["gt_b2"] = gt_b2
                return _cache["gt_b2"]

            def PB1():
                if "pb1" not in _cache:
                    pb1 = wpool.tile([P, NP * kcp], F32, name="pb1")
                    nc.sync.dma_start(
                        pb1[:], w["pb1"][:].rearrange("q (c p) -> p (q c)", p=P)
                    )
                    _cache["pb1"] = pb1
                return _cache["pb1"]

            def PB2ROW():
                if "pb2row" not in _cache:
                    pb2row = wpool.tile([1, NP * 2 * A], DT, name="pb2row")
                    nc.gpsimd.dma_start(
                        pb2row[:], w["pb2"][:].rearrange("p d -> (p d)").unsqueeze(0)
                    )
                    _cache["pb2row"] = pb2row
                return _cache["pb2row"]

            def VFB2ROW():
                if "vf_b2row" not in _cache:
                    vf_b2row_f = wpool.tile([1, VH2], F32, name="vf_b2row_f")
                    nc.sync.dma_start(vf_b2row_f[:], w["vf_b2"][:].unsqueeze(0))
                    vf_b2row = wpool.tile([1, VH2], DT, name="vf_b2row")
                    nc.vector.tensor_copy(vf_b2row[:], vf_b2row_f[:])
                    _cache["vf_b2row"] = vf_b2row
                return _cache["vf_b2row"]

            ones_f = wpool.tile([1, P], F32)
            nc.vector.memset(ones_f[:], 1.0)
            ones = wpool.tile([1, P], DT)
            nc.vector.tensor_copy(ones[:], ones_f[:])

            # dense layer in transposed layout:
            # out chunk mc = relu/act(sum_k W[:, k*cols+mc*P:...]^T @ x_k + b[mc])
            def dense_T(out_sb, x_chunks, w_sb, cols, b_sb, act_engines):
                mc_n = cols // P
                for mc in range(mc_n):
                    ps = pp_mm.tile([P, NT], F32, tag="mm", name="ps")
                    nk = len(x_chunks)
                    for ik, (xt, xo) in enumerate(x_chunks):
                        nc.tensor.matmul(
                            ps[:],
                            w_sb[:, ik * cols + mc * P : ik * cols + (mc + 1) * P],
                            xt[:, xo : xo + NT],
                            start=(ik == 0),
                            stop=(ik == nk - 1),
                        )
                    if act_engines[mc % len(act_engines)] == "act":
                        nc.scalar.activation(
                            out_sb[:, mc * NT : (mc + 1) * NT], ps[:], AF.Relu,
                            bias=b_sb[:, mc : mc + 1],
                        )
                    else:
                        nc.vector.tensor_scalar(
                            out_sb[:, mc * NT : (mc + 1) * NT], ps[:],
                            b_sb[:, mc : mc + 1], 0.0, ALU.add, ALU.max,
                        )

            # ---------- per batch tile ----------
            for it in range(NTILES):
                r0 = it * NT

                # load + transpose features
                fT = apool.tile([P, 5 * NT], DT, tag="fT", bufs=3, name="fT")
                if DT2:
                    for fk in range(5):
                        nc.sync.dma_start(
                            fT[:, fk * NT : (fk + 1) * NT],
                            feat_d[r0 : r0 + NT, fk * P : (fk + 1) * P],
                            transpose=True,
                        )
                else:
                    for cb in range(NB):
                        x_stage = apool.tile([P, F], DT, tag="xs", bufs=NB + 1,
                                             name="xs")
                        nc.gpsimd.dma_start(
                            x_stage[:], feat_d[r0 + cb * P : r0 + (cb + 1) * P, :]
                        )
                        for fk in range(5):
                            tp = pp_tp.tile([P, P], DT, tag="tp", name="tp")
                            nc.tensor.transpose(
                                tp[:], x_stage[:, fk * P : (fk + 1) * P], ident[:]
                            )
                            nc.vector.tensor_copy(
                                fT[:, fk * NT + cb * P : fk * NT + (cb + 1) * P],
                                tp[:],
                            )

                def fch(i):
                    return (fT, i * NT)

                # state encoder
                s_h1 = apool.tile([P, 4 * NT], DT, tag="h1big", bufs=big_bufs,
                                  name="s_h1")
                dense_T(s_h1, [fch(i) for i in range(4)], WT("se_w1", S, H1), H1, BCOL("se_b1", H1),
                        ["act", "act", "dve", "act"])
                s_emb = apool.tile([P, 2 * NT], DT, tag="emb", bufs=emb_bufs,
                                   name="s_emb")
                dense_T(s_emb, [(s_h1, i * NT) for i in range(4)], WT("se_w2", H1, H2), H2,
                        BCOL("se_b2", H2), ["act"])
                # goal encoder
                g_h1 = apool.tile([P, 4 * NT], DT, tag="h1big", bufs=big_bufs,
                                  name="g_h1")
                dense_T(g_h1, [fch(4)], WT("ge_w1", G, H1), H1, BCOL("ge_b1", H1), ["dve"])
                g_emb = apool.tile([P, 2 * NT], DT, tag="emb", bufs=emb_bufs,
                                   name="g_emb")
                dense_T(g_emb, [(g_h1, i * NT) for i in range(4)], WT("ge_w2", H1, H2), H2,
                        BCOL("ge_b2", H2), ["dve"])
                # gate
                gt_h = apool.tile([P, 2 * NT], DT, tag="emb", bufs=emb_bufs,
                                  name="gt_h")
                dense_T(gt_h, [(s_emb, 0), (s_emb, NT), (g_emb, 0), (g_emb, NT)],
                        WT("gt_w1", 2 * H2, H2), H2, BCOL("gt_b1", H2), ["dve", "act"])
                gps = pp_mm.tile([NP, NT], F32, tag="mm", name="gps")
                for ik in range(2):
                    nc.tensor.matmul(
                        gps[:], WT("gt_w2", H2, NP)[:, ik * NP : (ik + 1) * NP],
                        gt_h[:, ik * NT : (ik + 1) * NT],
                        start=(ik == 0), stop=(ik == 1),
                    )
                gwT = apool.tile([NP, NT], DT, tag="gwT", name="gwT")
                nc.scalar.activation(gwT[:], gps[:], AF.Sigmoid, bias=GTB2()[:, 0:1])
                gwB = apool.tile([P, NB * NP], F32, tag="gwB", name="gwB")
                for cb in range(NB):
                    tp = pp_tp.tile([P, NP], DT, tag="tp", name="tpg")
                    nc.tensor.transpose(
                        tp[:], gwT[:, cb * P : (cb + 1) * P], ident[0:NP, 0:NP]
                    )
                    nc.vector.tensor_copy(gwB[:, cb * NP : (cb + 1) * NP], tp[:])

                # primitive state encoder
                pse_h1 = apool.tile([P, 4 * NT], DT, tag="h1big", bufs=big_bufs,
                                    name="pse_h1")
                dense_T(pse_h1, [fch(i) for i in range(4)], WT("pse_w1", S, H1), H1, BCOL("pse_b1", H1),
                        ["act", "dve", "act", "act"])
                peT = apool.tile([P, 2 * NT], DT, tag="peT", bufs=2, name="peT")
                dense_T(peT, [(pse_h1, i * NT) for i in range(4)], WT("pse_w2", H1, H2),
                        H2, BCOL("pse_b2", H2), ["act"])

                # primitives + mixture accumulation
                inv_full = {}
                invmu_full = {}
                NG = 8  # primitives per psum group
                pw1, pw2 = PW()
                pb1 = PB1()
                pb2row = PB2ROW()
                for g in range(NP // NG):
                    hp_list = []
                    for pi in range(NG):
                        p = g * NG + pi
                        h_p = apool.tile([P, kcp * NT], DT, tag="hp",
                                         bufs=NG + 2, name="h_p")
                        for mc in range(kcp):
                            ps1 = pp_mm.tile([P, NT], F32, tag="mm", name="ps1")
                            for ik in range(kcp):
                                nc.tensor.matmul(
                                    ps1[:],
                                    pw1[:, (p * kcp + ik) * H2 + mc * P :
                                         (p * kcp + ik) * H2 + (mc + 1) * P],
                                    peT[:, ik * NT : (ik + 1) * NT],
                                    start=(ik == 0), stop=(ik == kcp - 1),
                                )
                            if (p + mc) % 2 == 0:
                                nc.scalar.activation(
                                    h_p[:, mc * NT : (mc + 1) * NT], ps1[:], AF.Relu,
                                    bias=pb1[:, p * kcp + mc : p * kcp + mc + 1],
                                )
                            else:
                                nc.vector.tensor_scalar(
                                    h_p[:, mc * NT : (mc + 1) * NT], ps1[:],
                                    pb1[:, p * kcp + mc : p * kcp + mc + 1],
                                    0.0, ALU.add, ALU.max,
                                )
                        hp_list.append(h_p)
                    for half in range((NB + 1) // 2):
                        cbs = [c for c in (2 * half, 2 * half + 1) if c < NB]
                        fps = {}
                        for cb in cbs:
                            fp = (pp_mm if DT2 else pp_flip).tile(
                                [P, NG * 2 * A], F32,
                                tag="mm" if DT2 else f"flip{cb % 2}", name="fp")
                            fps[cb] = fp
                            nc.tensor.matmul(
                                fp[:], ones[0:1, :],
                                pb2row[0:1, g * NG * 2 * A : (g + 1) * NG * 2 * A],
                                start=True, stop=False, skip_group_check=True,
                            )
                        for pi in range(NG):
                            p = g * NG + pi
                            h_p = hp_list[pi]
                            for cb in cbs:
                                for ik in range(kcp):
                                    nc.tensor.matmul(
                                        fps[cb][:, pi * 2 * A : (pi + 1) * 2 * A],
                                        h_p[:, ik * NT + cb * P :
                                            ik * NT + (cb + 1) * P],
                                        pw2[:, (p * kcp + ik) * 2 * A :
                                            (p * kcp + ik + 1) * 2 * A],
                                        start=False, stop=(ik == kcp - 1),
                                        skip_group_check=True,
                                    )
                        # evacuate group psum -> mixture partials
                        for cb in cbs:
                            grp = fps[cb][:].rearrange("q (p d) -> q p d", p=NG)
                            ls_ap = grp[:, :, A : 2 * A]
                            mu_ap = grp[:, :, 0:A]
                            if g == 0:
                                inv_f = apool.tile([P, NP * A], F32,
                                                   tag=f"inv{cb}", name="inv_f")
                                invmu_f = apool.tile([P, NP * A], F32,
                                                     tag=f"invmu{cb}",
                                                     name="invmu_f")
                                inv_full[cb] = inv_f
                                invmu_full[cb] = invmu_f
                            sl = slice(g * NG * A, (g + 1) * NG * A)
                            isig = apool.tile([P, NG * A], F32, tag=f"isig{cb}",
                                              name="isig")
                            nc.scalar.activation(
                                isig[:].rearrange("q (p d) -> q p d", p=NG),
                                ls_ap, AF.Exp, scale=-1.0,
                            )
                            gw_bc = (
                                gwB[:, cb * NP + g * NG : cb * NP + (g + 1) * NG]
                                .unsqueeze(2).broadcast_to([P, NG, A])
                            )
                            nc.vector.tensor_tensor(
                                inv_full[cb][:, sl].rearrange(
                                    "q (p d) -> q p d", p=NG),
                                isig[:].rearrange("q (p d) -> q p d", p=NG),
                                gw_bc, ALU.mult,
                            )
                            nc.vector.tensor_tensor(
                                invmu_full[cb][:, sl].rearrange(
                                    "q (p d) -> q p d", p=NG),
                                inv_full[cb][:, sl].rearrange(
                                    "q (p d) -> q p d", p=NG),
                                mu_ap, ALU.mult,
                            )

                # reduce over all 16 primitives, divide, store mean
                mean_sb = apool.tile([P, NB * A], F32, tag="mean_sb", name="mean_sb")
                for cb in range(NB):
                    den = apool.tile([P, A], F32, tag=f"den{cb}", name="den")
                    nmr = apool.tile([P, A], F32, tag=f"nmr{cb}", name="nmr")
                    inv_sw = inv_full[cb][:].rearrange(
                        "q (p a) -> q p a", p=NP).transpose([0, 2, 1])
                    invmu_sw = invmu_full[cb][:].rearrange(
                        "q (p a) -> q p a", p=NP).transpose([0, 2, 1])
                    nc.vector.tensor_reduce(
                        out=den[:], in_=inv_sw, op=ALU.add,
                        axis=mybir.AxisListType.X,
                    )
                    nc.vector.tensor_reduce(
                        out=nmr[:], in_=invmu_sw, op=ALU.add,
                        axis=mybir.AxisListType.X,
                    )
                    rden = apool.tile([P, A], F32, tag=f"rden{cb}", name="rden")
                    nc.vector.reciprocal(rden[:], den[:])
                    nc.vector.tensor_tensor(
                        mean_sb[:, cb * A : (cb + 1) * A], nmr[:], rden[:], ALU.mult)
                nc.sync.dma_start(
                    mean_d[r0 : r0 + NT, :].rearrange("(c p) a -> p c a", p=P),
                    mean_sb[:].rearrange("p (c a) -> p c a", c=NB),
                )

                # value net
                v_h1 = apool.tile([P, 8 * NT], DT, tag="v_h1", bufs=2, name="v_h1")
                dense_T(v_h1, [fch(i) for i in range(5)], WT("vf_w1", F, VH1),
                        VH1, BCOL("vf_b1", VH1),
                        ["act", "act", "dve", "act", "act", "dve", "act", "act"])
                for cb in range(NB):
                    vps = (pp_mm if DT2 else pp_flip).tile(
                        [P, VH2], F32,
                        tag="mm" if DT2 else f"flip{cb % 2}", name="vps")
                    nc.tensor.matmul(
                        vps[:], ones[0:1, :], VFB2ROW()[0:1, :],
                        start=True, stop=False, skip_group_check=True,
                    )
                    for ik in range(8):
                        nc.tensor.matmul(
                            vps[:],
                            v_h1[:, ik * NT + cb * P : ik * NT + (cb + 1) * P],
                            WT("vf_w2", VH1, VH2)[:, ik * VH2 : (ik + 1) * VH2],
                            start=False, stop=(ik == 7), skip_group_check=True,
                        )
                    val = apool.tile([P, VH2], F32, tag="val", name="val")
                    if cb % 2 == 0:
                        nc.scalar.activation(val[:], vps[:], AF.Relu)
                    else:
                        nc.vector.tensor_scalar(
                            val[:], vps[:], 0.0, None, ALU.max)
                    nc.sync.dma_start(
                        value_d[r0 + cb * P : r0 + (cb + 1) * P, :], val[:]
                    )

    nc.finalize()
    return nc


def kernel(**inputs):
    if not _NC_CACHE:
        _NC_CACHE.append(build())
    nc = _NC_CACHE[0]

    arrs = {
        k: np.ascontiguousarray(np.asarray(v, dtype=np.float32))
        for k, v in inputs.items()
    }
    feats = arrs.pop("features")
    if DT2:
        npdt = mybir.dt.np(DT)
        feats = np.ascontiguousarray(feats.astype(npdt))
        for k in list(arrs):
            if k[-1].isdigit() and "_b" not in k and not k.startswith("pb"):
                arrs[k] = np.ascontiguousarray(arrs[k].astype(npdt))
    in_maps = []
    for c in range(NCORES):
        m = dict(arrs)
        m["features"] = np.ascontiguousarray(feats[c * BC : (c + 1) * BC])
        in_maps.append(m)

    res = run_bass_kernel_spmd(nc, in_maps, core_ids=list(range(NCORES)))
    mean = np.concatenate([r["mean"] for r in res.results], axis=0)
    value = np.concatenate([r["value"] for r in res.results], axis=0)
    return (mean, value)


# revision 37
# speedup vs baseline: 1.0180x; 1.0180x over previous
"""Trainium2 Bass kernel for nn_MCPHiddenLayers (dense MLP mixture-of-primitives).

Data-parallel over 8 NeuronCores: batch 32768 is split into 8 shards of
4096 rows; all MLP parameters are replicated. Each core runs an identical
NEFF built with the Tile framework.

Layout strategy: activations are kept feature-major ("transposed",
[feat_part, batch_free]) so every layer's contraction dim lands on SBUF
partitions with no per-layer transposes; the input features are
transposed once on-chip via the PE array. Matmuls run in a reduced
precision dtype (float32r by default: single-pass fp32, ~tf32 accuracy)
with fp32 PSUM accumulation. The final primitive layer and the value-net
output layer run "flipped" (batch on PSUM partitions) so the Gaussian
mixture can use per-partition scalar broadcasts; their free-dim biases
are injected via a ones-row matmul that also initializes the PSUM bank.
"""
import os
import sys

sys.path.insert(0, "/opt/trn_rl_repo")

import numpy as np

import concourse.bacc as bacc
import concourse.mybir as mybir
import concourse.tile as tile
from concourse.bass_utils import run_bass_kernel_spmd
from concourse.masks import make_identity

# ---- problem shapes (hardcoded from the spec) ----
B = 32768
S = 512
G = 128
A = 32
NP = 16          # num_primitives
H1, H2 = 512, 256
VH1, VH2 = 1024, 512
F = S + G        # 640

NCORES = 8
BC = B // NCORES  # 4096 rows per core
P = 128

# ---- tuning knobs ----
NT = int(os.environ.get("KNT", "512"))        # batch tile (matmul moving dim)
_DT = os.environ.get("KDT", "f16")            # matmul operand dtype
NTILES = BC // NT
NB = NT // P      # batch chunks of 128 per tile

F32 = mybir.dt.float32
DT = {"f32r": mybir.dt.float32r, "bf16": mybir.dt.bfloat16,
      "f16": mybir.dt.float16, "f32": mybir.dt.float32}[_DT]
DT2 = mybir.dt.size(DT) == 2   # 2-byte path: host-cast features + DMA transpose
AF = mybir.ActivationFunctionType
ALU = mybir.AluOpType

_NC_CACHE = []


def build():
    nc = bacc.Bacc(None, target_bir_lowering=False)

    feat_d = nc.dram_tensor(
        "features", [BC, F], DT if DT2 else F32, kind="ExternalInput"
    )
    w = {}
    for name, shape in [
        ("se_w1", [S, H1]), ("se_b1", [H1]), ("se_w2", [H1, H2]), ("se_b2", [H2]),
        ("ge_w1", [G, H1]), ("ge_b1", [H1]), ("ge_w2", [H1, H2]), ("ge_b2", [H2]),
        ("gt_w1", [2 * H2, H2]), ("gt_b1", [H2]), ("gt_w2", [H2, NP]), ("gt_b2", [NP]),
        ("pse_w1", [S, H1]), ("pse_b1", [H1]), ("pse_w2", [H1, H2]), ("pse_b2", [H2]),
        ("pw1", [NP, H2, H2]), ("pb1", [NP, H2]),
        ("pw2", [NP, H2, 2 * A]), ("pb2", [NP, 2 * A]),
        ("vf_w1", [F, VH1]), ("vf_b1", [VH1]), ("vf_w2", [VH1, VH2]), ("vf_b2", [VH2]),
    ]:
        is_wt = name[-1].isdigit() and "_b" not in name and not name.startswith("pb")
        w[name] = nc.dram_tensor(
            name, shape, DT if (DT2 and is_wt) else F32, kind="ExternalInput"
        )
    mean_d = nc.dram_tensor("mean", [BC, A], F32, kind="ExternalOutput")
    value_d = nc.dram_tensor("value", [BC, VH2], F32, kind="ExternalOutput")

    big_bufs = 3 if DT != F32 and mybir.dt.size(DT) == 2 else 2
    emb_bufs = 4 if mybir.dt.size(DT) == 2 else 3

    with tile.TileContext(nc) as tc:
        with (
            tc.tile_pool(name="wpool", bufs=1) as wpool,
            tc.tile_pool(name="apool", bufs=2) as apool,
            tc.tile_pool(name="ptp", bufs=1 if DT2 else 2, space="PSUM") as pp_tp,
            tc.tile_pool(name="pmm", bufs=7 if DT2 else 3, space="PSUM") as pp_mm,
            tc.tile_pool(name="pflip", bufs=1, space="PSUM") as pp_flip,
        ):
            # ---------- static setup ----------
            ident_f = wpool.tile([P, P], F32)
            make_identity(nc, ident_f)
            ident = wpool.tile([P, P], DT)
            nc.vector.tensor_copy(ident[:], ident_f[:])

            wdma = nc.sync.dma_start if DT2 else nc.gpsimd.dma_start

            def load_w(name, rows, cols, dma=None):
                kc = rows // P
                sb = wpool.tile([P, kc * cols], DT, name=name)
                (dma or wdma)(
                    sb[:].rearrange("p (c m) -> p c m", c=kc),
                    w[name][:].rearrange("(c p) m -> p c m", p=P),
                )
                return sb

            def load_b(name, n):
                c = n // P
                sb = wpool.tile([P, c], F32, name=name)
                nc.gpsimd.dma_start(sb[:], w[name][:].rearrange("(c p) -> p c", p=P))
                return sb

            _cache = {}

            def WT(name, rows, cols, dma=None):
                if name not in _cache:
                    _cache[name] = load_w(name, rows, cols, dma)
                return _cache[name]

            def BCOL(name, n):
                key = "b:" + name
                if key not in _cache:
                    _cache[key] = load_b(name, n)
                return _cache[key]

            kcp = H2 // P  # 2

            def PW():
                if "pw" not in _cache:
                    pw1 = wpool.tile([P, NP * kcp * H2], DT, name="pw1")
                    pw2 = wpool.tile([P, NP * kcp * 2 * A], DT, name="pw2")
                    wdma(
                        pw1[:].rearrange("p (q c m) -> p q c m", q=NP, c=kcp),
                        w["pw1"][:].rearrange("q (c p) m -> p q c m", p=P),
                    )
                    wdma(
                        pw2[:].rearrange("p (q c m) -> p q c m", q=NP, c=kcp),
                        w["pw2"][:].rearrange("q (c p) m -> p q c m", p=P),
                    )
                    _cache["pw"] = (pw1, pw2)
                return _cache["pw"]

            def GTB2():
                if "gt_b2" not in _cache:
                    gt_b2 = wpool.tile([NP, 1], F32, name="gt_b2")
                    nc.sync.dma_start(gt_b2[:], w["gt_b2"][:].unsqueeze(1))
                    _cache["gt_b2"] = gt_b2
                return _cache["gt_b2"]

            def PB1():
                if "pb1" not in _cache:
                    pb1 = wpool.tile([P, NP * kcp], F32, name="pb1")
                    nc.sync.dma_start(
                        pb1[:], w["pb1"][:].rearrange("q (c p) -> p (q c)", p=P)
                    )
                    _cache["pb1"] = pb1
                return _cache["pb1"]

            def PB2ROW():
                if "pb2row" not in _cache:
                    pb2row = wpool.tile([1, NP * 2 * A], DT, name="pb2row")
                    nc.gpsimd.dma_start(
                        pb2row[:], w["pb2"][:].rearrange("p d -> (p d)").unsqueeze(0)
                    )
                    _cache["pb2row"] = pb2row
                return _cache["pb2row"]

            def VFB2ROW():
                if "vf_b2row" not in _cache:
                    vf_b2row_f = wpool.tile([1, VH2], F32, name="vf_b2row_f")
                    nc.sync.dma_start(vf_b2row_f[:], w["vf_b2"][:].unsqueeze(0))
                    vf_b2row = wpool.tile([1, VH2], DT, name="vf_b2row")
                    nc.vector.tensor_copy(vf_b2row[:], vf_b2row_f[:])
                    _cache["vf_b2row"] = vf_b2row
                return _cache["vf_b2row"]

            ones_f = wpool.tile([1, P], F32)
            nc.vector.memset(ones_f[:], 1.0)
            ones = wpool.tile([1, P], DT)
            nc.vector.tensor_copy(ones[:], ones_f[:])

            # dense layer in transposed layout:
            # out chunk mc = relu/act(sum_k W[:, k*cols+mc*P:...]^T @ x_k + b[mc])
            def dense_T(out_sb, x_chunks, w_sb, cols, b_sb, act_engines):
                mc_n = cols // P
                for mc in range(mc_n):
                    ps = pp_mm.tile([P, NT], F32, tag="mm", name="ps")
                    nk = len(x_chunks)
                    for ik, (xt, xo) in enumerate(x_chunks):
                        nc.tensor.matmul(
                            ps[:],
                            w_sb[:, ik * cols + mc * P : ik * cols + (mc + 1) * P],
                            xt[:, xo : xo + NT],
                            start=(ik == 0),
                            stop=(ik == nk - 1),
                        )
                    if act_engines[mc % len(act_engines)] == "act":
                        nc.scalar.activation(
                            out_sb[:, mc * NT : (mc + 1) * NT], ps[:], AF.Relu,
                            bias=b_sb[:, mc : mc + 1],
                        )
                    else:
                        nc.vector.tensor_scalar(
                            out_sb[:, mc * NT : (mc + 1) * NT], ps[:],
                            b_sb[:, mc : mc + 1], 0.0, ALU.add, ALU.max,
                        )

            # ---------- per batch tile ----------
            for it in range(NTILES):
                r0 = it * NT

                # load + transpose features
                fT = apool.tile([P, 5 * NT], DT, tag="fT", name="fT")
                if DT2:
                    for fk in range(5):
                        nc.sync.dma_start(
                            fT[:, fk * NT : (fk + 1) * NT],
                            feat_d[r0 : r0 + NT, fk * P : (fk + 1) * P],
                            transpose=True,
                        )
                else:
                    for cb in range(NB):
                        x_stage = apool.tile([P, F], DT, tag="xs", bufs=NB + 1,
                                             name="xs")
                        nc.gpsimd.dma_start(
                            x_stage[:], feat_d[r0 + cb * P : r0 + (cb + 1) * P, :]
                        )
                        for fk in range(5):
                            tp = pp_tp.tile([P, P], DT, tag="tp", name="tp")
                            nc.tensor.transpose(
                                tp[:], x_stage[:, fk * P : (fk + 1) * P], ident[:]
                            )
                            nc.vector.tensor_copy(
                                fT[:, fk * NT + cb * P : fk * NT + (cb + 1) * P],
                                tp[:],
                            )

                def fch(i):
                    return (fT, i * NT)

                # state encoder
                s_h1 = apool.tile([P, 4 * NT], DT, tag="h1big", bufs=big_bufs,
                                  name="s_h1")
                dense_T(s_h1, [fch(i) for i in range(4)], WT("se_w1", S, H1), H1, BCOL("se_b1", H1),
                        ["act", "act", "dve", "act"])
                s_emb = apool.tile([P, 2 * NT], DT, tag="emb", bufs=emb_bufs,
                                   name="s_emb")
                dense_T(s_emb, [(s_h1, i * NT) for i in range(4)], WT("se_w2", H1, H2), H2,
                        BCOL("se_b2", H2), ["act"])
                # goal encoder
                g_h1 = apool.tile([P, 4 * NT], DT, tag="h1big", bufs=big_bufs,
                                  name="g_h1")
                dense_T(g_h1, [fch(4)], WT("ge_w1", G, H1), H1, BCOL("ge_b1", H1), ["dve"])
                g_emb = apool.tile([P, 2 * NT], DT, tag="emb", bufs=emb_bufs,
                                   name="g_emb")
                dense_T(g_emb, [(g_h1, i * NT) for i in range(4)], WT("ge_w2", H1, H2), H2,
                        BCOL("ge_b2", H2), ["dve"])
                # gate
                gt_h = apool.tile([P, 2 * NT], DT, tag="emb", bufs=emb_bufs,
                                  name="gt_h")
                dense_T(gt_h, [(s_emb, 0), (s_emb, NT), (g_emb, 0), (g_emb, NT)],
                        WT("gt_w1", 2 * H2, H2), H2, BCOL("gt_b1", H2), ["dve", "act"])
                gps = pp_mm.tile([NP, NT], F32, tag="mm", name="gps")
                for ik in range(2):
                    nc.tensor.matmul(
                        gps[:], WT("gt_w2", H2, NP)[:, ik * NP : (ik + 1) * NP],
                        gt_h[:, ik * NT : (ik + 1) * NT],
                        start=(ik == 0), stop=(ik == 1),
                    )
                gwT = apool.tile([NP, NT], DT, tag="gwT", name="gwT")
                nc.scalar.activation(gwT[:], gps[:], AF.Sigmoid, bias=GTB2()[:, 0:1])
                gwB = apool.tile([P, NB * NP], F32, tag="gwB", name="gwB")
                for cb in range(NB):
                    tp = pp_tp.tile([P, NP], DT, tag="tp", name="tpg")
                    nc.tensor.transpose(
                        tp[:], gwT[:, cb * P : (cb + 1) * P], ident[0:NP, 0:NP]
                    )
                    nc.vector.tensor_copy(gwB[:, cb * NP : (cb + 1) * NP], tp[:])

                # primitive state encoder
                pse_h1 = apool.tile([P, 4 * NT], DT, tag="h1big", bufs=big_bufs,
                                    name="pse_h1")
                dense_T(pse_h1, [fch(i) for i in range(4)], WT("pse_w1", S, H1), H1, BCOL("pse_b1", H1),
                        ["act", "dve", "act", "act"])
                peT = apool.tile([P, 2 * NT], DT, tag="peT", bufs=2, name="peT")
                dense_T(peT, [(pse_h1, i * NT) for i in range(4)], WT("pse_w2", H1, H2),
                        H2, BCOL("pse_b2", H2), ["act"])

                # primitives + mixture accumulation
                inv_full = {}
                invmu_full = {}
                NG = 8  # primitives per psum group
                pw1, pw2 = PW()
                pb1 = PB1()
                pb2row = PB2ROW()
                for g in range(NP // NG):
                    hp_list = []
                    for pi in range(NG):
                        p = g * NG + pi
                        h_p = apool.tile([P, kcp * NT], DT, tag="hp",
                                         bufs=NG + 2, name="h_p")
                        for mc in range(kcp):
                            ps1 = pp_mm.tile([P, NT], F32, tag="mm", name="ps1")
                            for ik in range(kcp):
                                nc.tensor.matmul(
                                    ps1[:],
                                    pw1[:, (p * kcp + ik) * H2 + mc * P :
                                         (p * kcp + ik) * H2 + (mc + 1) * P],
                                    peT[:, ik * NT : (ik + 1) * NT],
                                    start=(ik == 0), stop=(ik == kcp - 1),
                                )
                            if (p + mc) % 2 == 0:
                                nc.scalar.activation(
                                    h_p[:, mc * NT : (mc + 1) * NT], ps1[:], AF.Relu,
                                    bias=pb1[:, p * kcp + mc : p * kcp + mc + 1],
                                )
                            else:
                                nc.vector.tensor_scalar(
                                    h_p[:, mc * NT : (mc + 1) * NT], ps1[:],
                                    pb1[:, p * kcp + mc : p * kcp + mc + 1],
                                    0.0, ALU.add, ALU.max,
                                )
                        hp_list.append(h_p)
                    for half in range((NB + 1) // 2):
                        cbs = [c for c in (2 * half, 2 * half + 1) if c < NB]
                        fps = {}
                        for cb in cbs:
                            fp = (pp_mm if DT2 else pp_flip).tile(
                                [P, NG * 2 * A], F32,
                                tag="mm" if DT2 else f"flip{cb % 2}", name="fp")
                            fps[cb] = fp
                            nc.tensor.matmul(
                                fp[:], ones[0:1, :],
                                pb2row[0:1, g * NG * 2 * A : (g + 1) * NG * 2 * A],
                                start=True, stop=False, skip_group_check=True,
                            )
                        for pi in range(NG):
                            p = g * NG + pi
                            h_p = hp_list[pi]
                            for cb in cbs:
                                for ik in range(kcp):
                                    nc.tensor.matmul(
                                        fps[cb][:, pi * 2 * A : (pi + 1) * 2 * A],
                                        h_p[:, ik * NT + cb * P :
                                            ik * NT + (cb + 1) * P],
                                        pw2[:, (p * kcp + ik) * 2 * A :
                                            (p * kcp + ik + 1) * 2 * A],
                                        start=False, stop=(ik == kcp - 1),
                                        skip_group_check=True,
                                    )
                        # evacuate group psum -> mixture partials
                        for cb in cbs:
                            grp = fps[cb][:].rearrange("q (p d) -> q p d", p=NG)
                            ls_ap = grp[:, :, A : 2 * A]
                            mu_ap = grp[:, :, 0:A]
                            if g == 0:
                                inv_f = apool.tile([P, NP * A], F32,
                                                   tag=f"inv{cb}", name="inv_f")
                                invmu_f = apool.tile([P, NP * A], F32,
                                                     tag=f"invmu{cb}",
                                                     name="invmu_f")
                                inv_full[cb] = inv_f
                                invmu_full[cb] = invmu_f
                            sl = slice(g * NG * A, (g + 1) * NG * A)
                            isig = apool.tile([P, NG * A], F32, tag=f"isig{cb}",
                                              name="isig")
                            nc.scalar.activation(
                                isig[:].rearrange("q (p d) -> q p d", p=NG),
                                ls_ap, AF.Exp, scale=-1.0,
                            )
                            gw_bc = (
                                gwB[:, cb * NP + g * NG : cb * NP + (g + 1) * NG]
                                .unsqueeze(2).broadcast_to([P, NG, A])
                            )
                            nc.vector.tensor_tensor(
                                inv_full[cb][:, sl].rearrange(
                                    "q (p d) -> q p d", p=NG),
                                isig[:].rearrange("q (p d) -> q p d", p=NG),
                                gw_bc, ALU.mult,
                            )
                            nc.vector.tensor_tensor(
                                invmu_full[cb][:, sl].rearrange(
                                    "q (p d) -> q p d", p=NG),
                                inv_full[cb][:, sl].rearrange(
                                    "q (p d) -> q p d", p=NG),
                                mu_ap, ALU.mult,
                            )

                # reduce over all 16 primitives, divide, store mean
                mean_sb = apool.tile([P, NB * A], F32, tag="mean_sb", name="mean_sb")
                for cb in range(NB):
                    den = apool.tile([P, A], F32, tag=f"den{cb}", name="den")
                    nmr = apool.tile([P, A], F32, tag=f"nmr{cb}", name="nmr")
                    inv_sw = inv_full[cb][:].rearrange(
                        "q (p a) -> q p a", p=NP).transpose([0, 2, 1])
                    invmu_sw = invmu_full[cb][:].rearrange(
                        "q (p a) -> q p a", p=NP).transpose([0, 2, 1])
                    nc.vector.tensor_reduce(
                        out=den[:], in_=inv_sw, op=ALU.add,
                        axis=mybir.AxisListType.X,
                    )
                    nc.vector.tensor_reduce(
                        out=nmr[:], in_=invmu_sw, op=ALU.add,
                        axis=mybir.AxisListType.X,
                    )
                    rden = apool.tile([P, A], F32, tag=f"rden{cb}", name="rden")
                    nc.vector.reciprocal(rden[:], den[:])
                    nc.vector.tensor_tensor(
                        mean_sb[:, cb * A : (cb + 1) * A], nmr[:], rden[:], ALU.mult)
                nc.sync.dma_start(
                    mean_d[r0 : r0 + NT, :].rearrange("(c p) a -> p c a", p=P),
                    mean_sb[:].rearrange("p (c a) -> p c a", c=NB),
                )

                # value net
                v_h1 = apool.tile([P, 8 * NT], DT, tag="v_h1", bufs=2, name="v_h1")
                dense_T(v_h1, [fch(i) for i in range(5)], WT("vf_w1", F, VH1),
                        VH1, BCOL("vf_b1", VH1),
                        ["act", "act", "dve", "act", "act", "dve", "act", "act"])
                for cb in range(NB):
                    vps = (pp_mm if DT2 else pp_flip).tile(
                        [P, VH2], F32,
                        tag="mm" if DT2 else f"flip{cb % 2}", name="vps")
                    nc.tensor.matmul(
                        vps[:], ones[0:1, :], VFB2ROW()[0:1, :],
                        start=True, stop=False, skip_group_check=True,
                    )
                    for ik in range(8):
                        nc.tensor.matmul(
                            vps[:],
                            v_h1[:, ik * NT + cb * P : ik * NT + (cb + 1) * P],
                            WT("vf_w2", VH1, VH2)[:, ik * VH2 : (ik + 1) * VH2],
                            start=False, stop=(ik == 7), skip_group_check=True,
                        )
                    val = apool.tile([P, VH2], F32, tag="val", name="val")
                    if cb % 2 == 0:
                        nc.scalar.activation(val[:], vps[:], AF.Relu)
                    else:
                        nc.vector.tensor_scalar(
                            val[:], vps[:], 0.0, None, ALU.max)
                    nc.sync.dma_start(
                        value_d[r0 + cb * P : r0 + (cb + 1) * P, :], val[:]
                    )

    nc.finalize()
    return nc


def kernel(**inputs):
    if not _NC_CACHE:
        _NC_CACHE.append(build())
    nc = _NC_CACHE[0]

    arrs = {
        k: np.ascontiguousarray(np.asarray(v, dtype=np.float32))
        for k, v in inputs.items()
    }
    feats = arrs.pop("features")
    if DT2:
        npdt = mybir.dt.np(DT)
        feats = np.ascontiguousarray(feats.astype(npdt))
        for k in list(arrs):
            if k[-1].isdigit() and "_b" not in k and not k.startswith("pb"):
                arrs[k] = np.ascontiguousarray(arrs[k].astype(npdt))
    in_maps = []
    for c in range(NCORES):
        m = dict(arrs)
        m["features"] = np.ascontiguousarray(feats[c * BC : (c + 1) * BC])
        in_maps.append(m)

    res = run_bass_kernel_spmd(nc, in_maps, core_ids=list(range(NCORES)))
    mean = np.concatenate([r["mean"] for r in res.results], axis=0)
    value = np.concatenate([r["value"] for r in res.results], axis=0)
    return (mean, value)


# revision 38
# speedup vs baseline: 1.0790x; 1.0599x over previous
"""Trainium2 Bass kernel for nn_MCPHiddenLayers (dense MLP mixture-of-primitives).

Data-parallel over 8 NeuronCores: batch 32768 is split into 8 shards of
4096 rows; all MLP parameters are replicated. Each core runs an identical
NEFF built with the Tile framework.

Layout strategy: activations are kept feature-major ("transposed",
[feat_part, batch_free]) so every layer's contraction dim lands on SBUF
partitions with no per-layer transposes; the input features are
transposed once on-chip via the PE array. Matmuls run in a reduced
precision dtype (float32r by default: single-pass fp32, ~tf32 accuracy)
with fp32 PSUM accumulation. The final primitive layer and the value-net
output layer run "flipped" (batch on PSUM partitions) so the Gaussian
mixture can use per-partition scalar broadcasts; their free-dim biases
are injected via a ones-row matmul that also initializes the PSUM bank.
"""
import os
import sys

sys.path.insert(0, "/opt/trn_rl_repo")

import numpy as np

import concourse.bacc as bacc
import concourse.mybir as mybir
import concourse.tile as tile
from concourse.bass_utils import run_bass_kernel_spmd
from concourse.masks import make_identity

# ---- problem shapes (hardcoded from the spec) ----
B = 32768
S = 512
G = 128
A = 32
NP = 16          # num_primitives
H1, H2 = 512, 256
VH1, VH2 = 1024, 512
F = S + G        # 640

NCORES = 8
BC = B // NCORES  # 4096 rows per core
P = 128

# ---- tuning knobs ----
NT = int(os.environ.get("KNT", "512"))        # batch tile (matmul moving dim)
_DT = os.environ.get("KDT", "f16")            # matmul operand dtype
NTILES = BC // NT
NB = NT // P      # batch chunks of 128 per tile

F32 = mybir.dt.float32
DT = {"f32r": mybir.dt.float32r, "bf16": mybir.dt.bfloat16,
      "f16": mybir.dt.float16, "f32": mybir.dt.float32}[_DT]
DT2 = mybir.dt.size(DT) == 2   # 2-byte path: host-cast features + DMA transpose
AF = mybir.ActivationFunctionType
ALU = mybir.AluOpType

_NC_CACHE = []


def build():
    nc = bacc.Bacc(None, target_bir_lowering=False)

    feat_d = nc.dram_tensor(
        "features", [BC, F], DT if DT2 else F32, kind="ExternalInput"
    )
    w = {}
    for name, shape in [
        ("se_w1", [S, H1]), ("se_b1", [H1]), ("se_w2", [H1, H2]), ("se_b2", [H2]),
        ("ge_w1", [G, H1]), ("ge_b1", [H1]), ("ge_w2", [H1, H2]), ("ge_b2", [H2]),
        ("gt_w1", [2 * H2, H2]), ("gt_b1", [H2]), ("gt_w2", [H2, NP]), ("gt_b2", [NP]),
        ("pse_w1", [S, H1]), ("pse_b1", [H1]), ("pse_w2", [H1, H2]), ("pse_b2", [H2]),
        ("pw1", [NP, H2, H2]), ("pb1", [NP, H2]),
        ("pw2", [NP, H2, 2 * A]), ("pb2", [NP, 2 * A]),
        ("vf_w1", [F, VH1]), ("vf_b1", [VH1]), ("vf_w2", [VH1, VH2]), ("vf_b2", [VH2]),
    ]:
        is_wt = name[-1].isdigit() and "_b" not in name and not name.startswith("pb")
        w[name] = nc.dram_tensor(
            name, shape, DT if (DT2 and is_wt) else F32, kind="ExternalInput"
        )
    mean_d = nc.dram_tensor("mean", [BC, A], F32, kind="ExternalOutput")
    value_d = nc.dram_tensor("value", [BC, VH2], F32, kind="ExternalOutput")

    big_bufs = 3 if DT != F32 and mybir.dt.size(DT) == 2 else 2
    emb_bufs = 4 if mybir.dt.size(DT) == 2 else 3

    with tile.TileContext(nc) as tc:
        with (
            tc.tile_pool(name="wpool", bufs=1) as wpool,
            tc.tile_pool(name="apool", bufs=2) as apool,
            tc.tile_pool(name="ptp", bufs=1 if DT2 else 2, space="PSUM") as pp_tp,
            tc.tile_pool(name="pmm", bufs=7 if DT2 else 3, space="PSUM") as pp_mm,
            tc.tile_pool(name="pflip", bufs=1, space="PSUM") as pp_flip,
        ):
            # ---------- static setup ----------
            ident_f = wpool.tile([P, P], F32)
            make_identity(nc, ident_f)
            ident = wpool.tile([P, P], DT)
            nc.vector.tensor_copy(ident[:], ident_f[:])

            wdma = nc.sync.dma_start if DT2 else nc.gpsimd.dma_start

            def load_w(name, rows, cols, dma=None):
                kc = rows // P
                sb = wpool.tile([P, kc * cols], DT, name=name)
                (dma or wdma)(
                    sb[:].rearrange("p (c m) -> p c m", c=kc),
                    w[name][:].rearrange("(c p) m -> p c m", p=P),
                )
                return sb

            def load_b(name, n):
                c = n // P
                sb = wpool.tile([P, c], F32, name=name)
                nc.gpsimd.dma_start(sb[:], w[name][:].rearrange("(c p) -> p c", p=P))
                return sb

            _cache = {}

            def WT(name, rows, cols, dma=None):
                if name not in _cache:
                    _cache[name] = load_w(name, rows, cols, dma)
                return _cache[name]

            def BCOL(name, n):
                key = "b:" + name
                if key not in _cache:
                    _cache[key] = load_b(name, n)
                return _cache[key]

            kcp = H2 // P  # 2

            def PW():
                if "pw" not in _cache:
                    pw1 = wpool.tile([P, NP * kcp * H2], DT, name="pw1")
                    pw2 = wpool.tile([P, NP * kcp * 2 * A], DT, name="pw2")
                    wdma(
                        pw1[:].rearrange("p (q c m) -> p q c m", q=NP, c=kcp),
                        w["pw1"][:].rearrange("q (c p) m -> p q c m", p=P),
                    )
                    wdma(
                        pw2[:].rearrange("p (q c m) -> p q c m", q=NP, c=kcp),
                        w["pw2"][:].rearrange("q (c p) m -> p q c m", p=P),
                    )
                    _cache["pw"] = (pw1, pw2)
                return _cache["pw"]

            def GTB2():
                if "gt_b2" not in _cache:
                    gt_b2 = wpool.tile([NP, 1], F32, name="gt_b2")
                    nc.sync.dma_start(gt_b2[:], w["gt_b2"][:].unsqueeze(1))
                    _cache["gt_b2"] = gt_b2
                return _cache["gt_b2"]

            def PB1():
                if "pb1" not in _cache:
                    pb1 = wpool.tile([P, NP * kcp], F32, name="pb1")
                    nc.sync.dma_start(
                        pb1[:], w["pb1"][:].rearrange("q (c p) -> p (q c)", p=P)
                    )
                    _cache["pb1"] = pb1
                return _cache["pb1"]

            def PB2ROW():
                if "pb2row" not in _cache:
                    pb2row = wpool.tile([1, NP * 2 * A], DT, name="pb2row")
                    nc.gpsimd.dma_start(
                        pb2row[:], w["pb2"][:].rearrange("p d -> (p d)").unsqueeze(0)
                    )
                    _cache["pb2row"] = pb2row
                return _cache["pb2row"]

            def VFB2ROW():
                if "vf_b2row" not in _cache:
                    vf_b2row_f = wpool.tile([1, VH2], F32, name="vf_b2row_f")
                    nc.sync.dma_start(vf_b2row_f[:], w["vf_b2"][:].unsqueeze(0))
                    vf_b2row = wpool.tile([1, VH2], DT, name="vf_b2row")
                    nc.vector.tensor_copy(vf_b2row[:], vf_b2row_f[:])
                    _cache["vf_b2row"] = vf_b2row
                return _cache["vf_b2row"]

            ones_f = wpool.tile([1, P], F32)
            nc.vector.memset(ones_f[:], 1.0)
            ones = wpool.tile([1, P], DT)
            nc.vector.tensor_copy(ones[:], ones_f[:])

            # dense layer in transposed layout:
            # out chunk mc = relu/act(sum_k W[:, k*cols+mc*P:...]^T @ x_k + b[mc])
            def dense_T(out_sb, x_chunks, w_sb, cols, b_sb, act_engines):
                mc_n = cols // P
                for mc in range(mc_n):
                    ps = pp_mm.tile([P, NT], F32, tag="mm", name="ps")
                    nk = len(x_chunks)
                    for ik, (xt, xo) in enumerate(x_chunks):
                        nc.tensor.matmul(
                            ps[:],
                            w_sb[:, ik * cols + mc * P : ik * cols + (mc + 1) * P],
                            xt[:, xo : xo + NT],
                            start=(ik == 0),
                            stop=(ik == nk - 1),
                        )
                    if act_engines[mc % len(act_engines)] == "act":
                        nc.scalar.activation(
                            out_sb[:, mc * NT : (mc + 1) * NT], ps[:], AF.Relu,
                            bias=b_sb[:, mc : mc + 1],
                        )
                    else:
                        nc.vector.tensor_scalar(
                            out_sb[:, mc * NT : (mc + 1) * NT], ps[:],
                            b_sb[:, mc : mc + 1], 0.0, ALU.add, ALU.max,
                        )

            # ---------- per batch tile ----------
            for it in range(NTILES):
                r0 = it * NT

                # load + transpose features
                fT = apool.tile([P, 5 * NT], DT, tag="fT", name="fT")
                if DT2:
                    for fk in range(5):
                        nc.sync.dma_start(
                            fT[:, fk * NT : (fk + 1) * NT],
                            feat_d[r0 : r0 + NT, fk * P : (fk + 1) * P],
                            transpose=True,
                        )
                else:
                    for cb in range(NB):
                        x_stage = apool.tile([P, F], DT, tag="xs", bufs=NB + 1,
                                             name="xs")
                        nc.gpsimd.dma_start(
                            x_stage[:], feat_d[r0 + cb * P : r0 + (cb + 1) * P, :]
                        )
                        for fk in range(5):
                            tp = pp_tp.tile([P, P], DT, tag="tp", name="tp")
                            nc.tensor.transpose(
                                tp[:], x_stage[:, fk * P : (fk + 1) * P], ident[:]
                            )
                            nc.vector.tensor_copy(
                                fT[:, fk * NT + cb * P : fk * NT + (cb + 1) * P],
                                tp[:],
                            )

                def fch(i):
                    return (fT, i * NT)

                # state encoder
                s_h1 = apool.tile([P, 4 * NT], DT, tag="h1big", bufs=big_bufs,
                                  name="s_h1")
                dense_T(s_h1, [fch(i) for i in range(4)], WT("se_w1", S, H1), H1, BCOL("se_b1", H1),
                        ["act", "act", "dve", "act"])
                s_emb = apool.tile([P, 2 * NT], DT, tag="emb", bufs=emb_bufs,
                                   name="s_emb")
                dense_T(s_emb, [(s_h1, i * NT) for i in range(4)], WT("se_w2", H1, H2), H2,
                        BCOL("se_b2", H2), ["act"])
                # goal encoder
                g_h1 = apool.tile([P, 4 * NT], DT, tag="h1big", bufs=big_bufs,
                                  name="g_h1")
                dense_T(g_h1, [fch(4)], WT("ge_w1", G, H1), H1, BCOL("ge_b1", H1), ["dve"])
                g_emb = apool.tile([P, 2 * NT], DT, tag="emb", bufs=emb_bufs,
                                   name="g_emb")
                dense_T(g_emb, [(g_h1, i * NT) for i in range(4)], WT("ge_w2", H1, H2), H2,
                        BCOL("ge_b2", H2), ["dve"])
                # gate
                gt_h = apool.tile([P, 2 * NT], DT, tag="emb", bufs=emb_bufs,
                                  name="gt_h")
                dense_T(gt_h, [(s_emb, 0), (s_emb, NT), (g_emb, 0), (g_emb, NT)],
                        WT("gt_w1", 2 * H2, H2), H2, BCOL("gt_b1", H2), ["dve", "act"])
                gps = pp_mm.tile([NP, NT], F32, tag="mm", name="gps")
                for ik in range(2):
                    nc.tensor.matmul(
                        gps[:], WT("gt_w2", H2, NP)[:, ik * NP : (ik + 1) * NP],
                        gt_h[:, ik * NT : (ik + 1) * NT],
                        start=(ik == 0), stop=(ik == 1),
                    )
                gwT = apool.tile([NP, NT], DT, tag="gwT", name="gwT")
                nc.scalar.activation(gwT[:], gps[:], AF.Sigmoid, bias=GTB2()[:, 0:1])
                gwB = apool.tile([P, NB * NP], F32, tag="gwB", name="gwB")
                for cb in range(NB):
                    tp = pp_tp.tile([P, NP], DT, tag="tp", name="tpg")
                    nc.tensor.transpose(
                        tp[:], gwT[:, cb * P : (cb + 1) * P], ident[0:NP, 0:NP]
                    )
                    nc.vector.tensor_copy(gwB[:, cb * NP : (cb + 1) * NP], tp[:])

                # primitive state encoder
                pse_h1 = apool.tile([P, 4 * NT], DT, tag="h1big", bufs=big_bufs,
                                    name="pse_h1")
                dense_T(pse_h1, [fch(i) for i in range(4)], WT("pse_w1", S, H1), H1, BCOL("pse_b1", H1),
                        ["act", "dve", "act", "act"])
                peT = apool.tile([P, 2 * NT], DT, tag="peT", bufs=2, name="peT")
                dense_T(peT, [(pse_h1, i * NT) for i in range(4)], WT("pse_w2", H1, H2),
                        H2, BCOL("pse_b2", H2), ["act"])

                # value net
                v_h1 = apool.tile([P, 8 * NT], DT, tag="v_h1", bufs=2, name="v_h1")
                dense_T(v_h1, [fch(i) for i in range(5)], WT("vf_w1", F, VH1),
                        VH1, BCOL("vf_b1", VH1),
                        ["act", "act", "dve", "act", "act", "dve", "act", "act"])

                # primitives + mixture accumulation
                inv_full = {}
                invmu_full = {}
                NG = 8  # primitives per psum group
                pw1, pw2 = PW()
                pb1 = PB1()
                pb2row = PB2ROW()
                for g in range(NP // NG):
                    hp_list = []
                    for pi in range(NG):
                        p = g * NG + pi
                        h_p = apool.tile([P, kcp * NT], DT, tag="hp",
                                         bufs=NG + 2, name="h_p")
                        for mc in range(kcp):
                            ps1 = pp_mm.tile([P, NT], F32, tag="mm", name="ps1")
                            for ik in range(kcp):
                                nc.tensor.matmul(
                                    ps1[:],
                                    pw1[:, (p * kcp + ik) * H2 + mc * P :
                                         (p * kcp + ik) * H2 + (mc + 1) * P],
                                    peT[:, ik * NT : (ik + 1) * NT],
                                    start=(ik == 0), stop=(ik == kcp - 1),
                                )
                            if (p + mc) % 2 == 0:
                                nc.scalar.activation(
                                    h_p[:, mc * NT : (mc + 1) * NT], ps1[:], AF.Relu,
                                    bias=pb1[:, p * kcp + mc : p * kcp + mc + 1],
                                )
                            else:
                                nc.vector.tensor_scalar(
                                    h_p[:, mc * NT : (mc + 1) * NT], ps1[:],
                                    pb1[:, p * kcp + mc : p * kcp + mc + 1],
                                    0.0, ALU.add, ALU.max,
                                )
                        hp_list.append(h_p)
                    for half in range((NB + 1) // 2):
                        cbs = [c for c in (2 * half, 2 * half + 1) if c < NB]
                        fps = {}
                        for cb in cbs:
                            fp = (pp_mm if DT2 else pp_flip).tile(
                                [P, NG * 2 * A], F32,
                                tag="mm" if DT2 else f"flip{cb % 2}", name="fp")
                            fps[cb] = fp
                            nc.tensor.matmul(
                                fp[:], ones[0:1, :],
                                pb2row[0:1, g * NG * 2 * A : (g + 1) * NG * 2 * A],
                                start=True, stop=False, skip_group_check=True,
                            )
                        for pi in range(NG):
                            p = g * NG + pi
                            h_p = hp_list[pi]
                            for cb in cbs:
                                for ik in range(kcp):
                                    nc.tensor.matmul(
                                        fps[cb][:, pi * 2 * A : (pi + 1) * 2 * A],
                                        h_p[:, ik * NT + cb * P :
                                            ik * NT + (cb + 1) * P],
                                        pw2[:, (p * kcp + ik) * 2 * A :
                                            (p * kcp + ik + 1) * 2 * A],
                                        start=False, stop=(ik == kcp - 1),
                                        skip_group_check=True,
                                    )
                        # evacuate group psum -> mixture partials
                        for cb in cbs:
                            grp = fps[cb][:].rearrange("q (p d) -> q p d", p=NG)
                            ls_ap = grp[:, :, A : 2 * A]
                            mu_ap = grp[:, :, 0:A]
                            if g == 0:
                                inv_f = apool.tile([P, NP * A], F32,
                                                   tag=f"inv{cb}", name="inv_f")
                                invmu_f = apool.tile([P, NP * A], F32,
                                                     tag=f"invmu{cb}",
                                                     name="invmu_f")
                                inv_full[cb] = inv_f
                                invmu_full[cb] = invmu_f
                            sl = slice(g * NG * A, (g + 1) * NG * A)
                            isig = apool.tile([P, NG * A], F32, tag=f"isig{cb}",
                                              name="isig")
                            nc.scalar.activation(
                                isig[:].rearrange("q (p d) -> q p d", p=NG),
                                ls_ap, AF.Exp, scale=-1.0,
                            )
                            gw_bc = (
                                gwB[:, cb * NP + g * NG : cb * NP + (g + 1) * NG]
                                .unsqueeze(2).broadcast_to([P, NG, A])
                            )
                            nc.vector.tensor_tensor(
                                inv_full[cb][:, sl].rearrange(
                                    "q (p d) -> q p d", p=NG),
                                isig[:].rearrange("q (p d) -> q p d", p=NG),
                                gw_bc, ALU.mult,
                            )
                            nc.vector.tensor_tensor(
                                invmu_full[cb][:, sl].rearrange(
                                    "q (p d) -> q p d", p=NG),
                                inv_full[cb][:, sl].rearrange(
                                    "q (p d) -> q p d", p=NG),
                                mu_ap, ALU.mult,
                            )

                # reduce over all 16 primitives, divide, store mean
                mean_sb = apool.tile([P, NB * A], F32, tag="mean_sb", name="mean_sb")
                for cb in range(NB):
                    den = apool.tile([P, A], F32, tag=f"den{cb}", name="den")
                    nmr = apool.tile([P, A], F32, tag=f"nmr{cb}", name="nmr")
                    inv_sw = inv_full[cb][:].rearrange(
                        "q (p a) -> q p a", p=NP).transpose([0, 2, 1])
                    invmu_sw = invmu_full[cb][:].rearrange(
                        "q (p a) -> q p a", p=NP).transpose([0, 2, 1])
                    nc.vector.tensor_reduce(
                        out=den[:], in_=inv_sw, op=ALU.add,
                        axis=mybir.AxisListType.X,
                    )
                    nc.vector.tensor_reduce(
                        out=nmr[:], in_=invmu_sw, op=ALU.add,
                        axis=mybir.AxisListType.X,
                    )
                    rden = apool.tile([P, A], F32, tag=f"rden{cb}", name="rden")
                    nc.vector.reciprocal(rden[:], den[:])
                    nc.vector.tensor_tensor(
                        mean_sb[:, cb * A : (cb + 1) * A], nmr[:], rden[:], ALU.mult)
                nc.sync.dma_start(
                    mean_d[r0 : r0 + NT, :].rearrange("(c p) a -> p c a", p=P),
                    mean_sb[:].rearrange("p (c a) -> p c a", c=NB),
                )

                for cb in range(NB):
                    vps = (pp_mm if DT2 else pp_flip).tile(
                        [P, VH2], F32,
                        tag="mm" if DT2 else f"flip{cb % 2}", name="vps")
                    nc.tensor.matmul(
                        vps[:], ones[0:1, :], VFB2ROW()[0:1, :],
                        start=True, stop=False, skip_group_check=True,
                    )
                    for ik in range(8):
                        nc.tensor.matmul(
                            vps[:],
                            v_h1[:, ik * NT + cb * P : ik * NT + (cb + 1) * P],
                            WT("vf_w2", VH1, VH2)[:, ik * VH2 : (ik + 1) * VH2],
                            start=False, stop=(ik == 7), skip_group_check=True,
                        )
                    val = apool.tile([P, VH2], F32, tag="val", name="val")
                    if cb % 2 == 0:
                        nc.scalar.activation(val[:], vps[:], AF.Relu)
                    else:
                        nc.vector.tensor_scalar(
                            val[:], vps[:], 0.0, None, ALU.max)
                    nc.sync.dma_start(
                        value_d[r0 + cb * P : r0 + (cb + 1) * P, :], val[:]
                    )

    nc.finalize()
    return nc


def kernel(**inputs):
    if not _NC_CACHE:
        _NC_CACHE.append(build())
    nc = _NC_CACHE[0]

    arrs = {
        k: np.ascontiguousarray(np.asarray(v, dtype=np.float32))
        for k, v in inputs.items()
    }
    feats = arrs.pop("features")
    if DT2:
        npdt = mybir.dt.np(DT)
        feats = np.ascontiguousarray(feats.astype(npdt))
        for k in list(arrs):
            if k[-1].isdigit() and "_b" not in k and not k.startswith("pb"):
                arrs[k] = np.ascontiguousarray(arrs[k].astype(npdt))
    in_maps = []
    for c in range(NCORES):
        m = dict(arrs)
        m["features"] = np.ascontiguousarray(feats[c * BC : (c + 1) * BC])
        in_maps.append(m)

    res = run_bass_kernel_spmd(nc, in_maps, core_ids=list(range(NCORES)))
    mean = np.concatenate([r["mean"] for r in res.results], axis=0)
    value = np.concatenate([r["value"] for r in res.results], axis=0)
    return (mean, value)
